# revision 7
# baseline (speedup 1.0000x reference)
"""Trainium2 Bass kernel for nn_CustomLossMinMax.

Computes, over full inputs pos_outputs [N,L], neg_outputs [M,L], p [N,L]
(N=M=8192, L=2048, f32):

    wpos[i]   = sum_l pos[i,l] * p[i,l]
    negmax[j] = max_l neg[j,l]
    out       = sum_ij relu(1 - wpos[i] + negmax[j]) / (N*M)

Sharding (8 cores): rows of pos/p and rows of neg are split 1024/core.
Each core computes its wpos shard and negmax shard, AllGathers the tiny
(1 + negmax) vector in bf16 (2 KiB/core), broadcasts it across
partitions, then accumulates the pairwise hinge for its own 1024 i-rows
against all 8192 j's. Per-core partial sums [128, 8] are summed on the
host (the scalar all-reduce step) and scaled by 1/(N*M).

Schedule per core (DMA of 24 MiB inputs is the roofline):
  - Input DMA is split across both HWDGE rings (sync + scalar), with all
    neg tiles enqueued first on both rings so negmax can trigger the
    AllGather at ~15 us. neg pool bufs=T so no slot-wait reorders DMAs.
  - gpsimd's SWDGE carries the collective bounce + the partition
    broadcast (single DMA, step-0 partition source).
  - DVE: row-max (8x) + fused -(pos*p) row-sum (8x) + 2 pairwise tiles.
  - ACT: 6 pairwise tiles: Relu(bcast + (-wpos)) with accumulation.

All i/j orderings inside the kernel are permutations of the reference
ordering; the final scalar sum is permutation-invariant. The bf16
transport of (1+negmax) perturbs the result by ~5e-6 relative.
"""
import sys
import numpy as np

for _p in ("/opt/trn_rl_repo", "/root/.axon_site/_ro/trn_rl_repo"):
    if _p not in sys.path:
        sys.path.insert(0, _p)

from concourse import bacc, mybir, tile  # noqa: E402
from concourse import bass_utils  # noqa: E402

N_CORES = 8
N, M, L = 8192, 8192, 2048
ROWS = N // N_CORES          # 1024 rows per core for pos/p and neg
T = ROWS // 128              # 8 row-tiles of 128 partitions per core
F32 = mybir.dt.float32
BF16 = mybir.dt.bfloat16

ACT_TILES = 6                # pairwise tiles on ScalarE
DVE_TILES = 2                # pairwise tiles on VectorE

_cache = {}


def _build():
    nc = bacc.Bacc("TRN2", target_bir_lowering=False, debug=False,
                   enable_asserts=True, num_devices=N_CORES)
    pos = nc.dram_tensor("pos", [ROWS, L], F32, kind="ExternalInput").ap()
    p = nc.dram_tensor("p", [ROWS, L], F32, kind="ExternalInput").ap()
    neg = nc.dram_tensor("neg", [ROWS, L], F32, kind="ExternalInput").ap()
    out = nc.dram_tensor("partial", [128, T], F32, kind="ExternalOutput").ap()

    pos_t = pos.rearrange("(t p) l -> t p l", p=128)
    p_t = p.rearrange("(t p) l -> t p l", p=128)
    neg_t = neg.rearrange("(t p) l -> t p l", p=128)

    with tile.TileContext(nc) as tc:
        with tc.tile_pool(name="negp", bufs=T) as neg_pool, \
             tc.tile_pool(name="posp", bufs=3) as pos_pool, \
             tc.tile_pool(name="scrp", bufs=2) as scr_pool, \
             tc.tile_pool(name="big", bufs=1) as big_pool, \
             tc.tile_pool(name="small", bufs=1) as small_pool, \
             tc.tile_pool(name="dram", bufs=1, space="DRAM") as dpool:

            # ---- Phase 1: neg DMA first on BOTH rings, then row-max ----
            negmax_sb = small_pool.tile([128, T], F32)
            ntiles = []
            for t in range(T):
                ntile = neg_pool.tile([128, L], F32, tag="neg")
                ring = nc.sync if t % 2 == 0 else nc.scalar
                ring.dma_start(ntile[:], neg_t[t])
                ntiles.append(ntile)
            for t in range(T):
                nc.vector.tensor_reduce(negmax_sb[:, t:t + 1], ntiles[t][:],
                                        axis=mybir.AxisListType.X,
                                        op=mybir.AluOpType.max)

            # fold the hinge's "+1" in and convert to bf16 for transport
            negmax1_bf = small_pool.tile([128, T], BF16)
            nc.vector.tensor_scalar_add(negmax1_bf[:], negmax_sb[:], 1.0)

            # ---- Phase 2: AllGather (1 + negmax), all on gpsimd SWDGE --
            cc_in = dpool.tile([128, T], BF16)
            cc_out = dpool.tile([128 * N_CORES, T], BF16)
            nc.gpsimd.dma_start(cc_in[:], negmax1_bf[:])
            nc.gpsimd.collective_compute(
                "AllGather",
                mybir.AluOpType.bypass,
                ins=[cc_in[:].opt()],
                outs=[cc_out[:].opt()],
                replica_groups=[list(range(N_CORES))],
            )
            # one DMA broadcasts the gathered (permuted) row to all
            # 128 partitions; descriptors fan out across SDMA engines
            bcast = big_pool.tile([128, M], BF16, tag="bcast")
            cc_row = cc_out[:].rearrange("a b -> (a b)").rearrange(
                "(a b) -> a b", a=1)
            nc.gpsimd.dma_start(bcast[:], cc_row.to_broadcast((128, M)))

            # ---- Phase 3: a = -wpos per row-tile (fused on DVE) --------
            # scalar_tensor_tensor: out = (pos * -1) * p, accum = sum(out)
            # pos on the sync ring, p on the scalar ring — both rings
            # drain their neg tiles first (FIFO per ring).
            a_sb = small_pool.tile([128, T], F32)
            for t in range(T):
                ptile = pos_pool.tile([128, L], F32, tag="pos")
                wtile = pos_pool.tile([128, L], F32, tag="p")
                nc.sync.dma_start(ptile[:], pos_t[t])
                nc.scalar.dma_start(wtile[:], p_t[t])
                scr = scr_pool.tile([128, L], BF16, tag="wpos_scr")
                nc.vector.scalar_tensor_tensor(
                    out=scr[:], in0=ptile[:], scalar=-1.0, in1=wtile[:],
                    op0=mybir.AluOpType.mult, op1=mybir.AluOpType.mult,
                    accum_out=a_sb[:, t:t + 1])

            # ---- Phase 4: pairwise hinge on ACT + DVE + GPSIMD ---------
            acc = small_pool.tile([128, T], F32)
            act_scr = big_pool.tile([128, M], BF16, tag="act_scr")
            dve_scr = big_pool.tile([128, M], BF16, tag="dve_scr")
            zeros = small_pool.tile([128, 1], BF16)
            nc.vector.memset(zeros[:], 0.0)
            zeros_b = zeros[:].broadcast_to((128, M))
            for t in range(T):
                if t < ACT_TILES:
                    nc.scalar.activation(
                        act_scr[:], bcast[:],
                        mybir.ActivationFunctionType.Relu,
                        bias=a_sb[:, t:t + 1], scale=1.0,
                        accum_out=acc[:, t:t + 1])
                else:
                    nc.vector.scalar_tensor_tensor(
                        out=dve_scr[:], in0=bcast[:],
                        scalar=a_sb[:, t:t + 1], in1=zeros_b,
                        op0=mybir.AluOpType.add, op1=mybir.AluOpType.max,
                        accum_out=acc[:, t:t + 1])

            nc.sync.dma_start(out, acc[:])
    nc.compile()
    return nc


def kernel(pos_outputs: np.ndarray, neg_outputs: np.ndarray,
           p: np.ndarray) -> np.ndarray:
    if "nc" not in _cache:
        _cache["nc"] = _build()
    nc = _cache["nc"]

    pos_outputs = np.ascontiguousarray(pos_outputs, dtype=np.float32)
    neg_outputs = np.ascontiguousarray(neg_outputs, dtype=np.float32)
    p = np.ascontiguousarray(p, dtype=np.float32)

    in_maps = []
    for c in range(N_CORES):
        sl = slice(c * ROWS, (c + 1) * ROWS)
        in_maps.append({
            "pos": pos_outputs[sl],
            "p": p[sl],
            "neg": neg_outputs[sl],
        })
    res = bass_utils.run_bass_kernel_spmd(nc, in_maps,
                                          core_ids=list(range(N_CORES)))
    total = 0.0
    for c in range(N_CORES):
        total += res.results[c]["partial"].astype(np.float64).sum()
    return np.asarray(total / (float(N) * float(M)), dtype=np.float32)


# revision 17
# speedup vs baseline: 1.4432x; 1.4432x over previous
"""Trainium2 Bass kernel for nn_CustomLossMinMax.

Computes, over full inputs pos_outputs [N,L], neg_outputs [M,L], p [N,L]
(N=M=8192, L=2048, f32):

    wpos[i]   = sum_l pos[i,l] * p[i,l]
    negmax[j] = max_l neg[j,l]
    out       = sum_ij relu(1 - wpos[i] + negmax[j]) / (N*M)

Sharding (8 cores): rows of pos/p and rows of neg are split 1024/core.
Each core computes its wpos shard and negmax shard, AllGathers the tiny
(1 + negmax) vector in bf16, replicates it across partitions, then
accumulates the pairwise hinge for its own 1024 i-rows against all 8192
j's. Per-core partial sums [128, 16] are summed on the host (the scalar
all-reduce step) and scaled by 1/(N*M).

Schedule per core (DMA of 24 MiB inputs is the roofline; the AllGather
start also absorbs the unavoidable inter-core launch skew):
  - neg tiles stream first (2-deep staggered DMA chain across both
    HWDGE rings); pos/p DMAs carry dep edges on all neg DMAs so the
    AllGather triggers as early as possible.
  - The negmax AllGather is split in TWO: AG1 ships tiles 0-3 as soon
    as they are reduced (~30 us), AG2 ships tiles 4-7. AG1 absorbs the
    inter-core start skew; AG2 pays only the mesh latency. Pairwise
    work on the first 4096 j's overlaps AG2 and the pos/p stream.
  - Partition broadcast per half: TensorE ones-matmuls (ones[1,128].T @
    negrow[1,512]) fill PSUM, then ONE ScalarE copy bridges PSUM->SBUF
    (PSUM readers serialize, so a single reader; ACT and DVE then run
    the pairwise in parallel from SBUF).
  - A tiny gpsimd DMA at kernel start pre-loads the Q7 SWDGE ucode so
    the collective bounce DMAs don't pay its ~8 us cold start.
  - DVE: row-max (8x), fused -(pos*p) row-sum (8x), 8 pairwise units.
  - ACT: 2 broadcast copies + 8 pairwise units with accumulation.

All i/j orderings inside the kernel are permutations of the reference
ordering; the final scalar sum is permutation-invariant. The bf16
transport of (1+negmax) perturbs the result by ~5e-6 relative.
"""
import sys
import numpy as np

for _p in ("/opt/trn_rl_repo", "/root/.axon_site/_ro/trn_rl_repo"):
    if _p not in sys.path:
        sys.path.insert(0, _p)

from concourse import bacc, mybir, tile  # noqa: E402
from concourse import bass_utils  # noqa: E402
from concourse.tile_rust import add_dep_helper  # noqa: E402

N_CORES = 8
N, M, L = 8192, 8192, 2048
ROWS = N // N_CORES          # 1024 rows per core for pos/p and neg
T = ROWS // 128              # 8 row-tiles of 128 partitions per core
TH = T // 2                  # tiles per AllGather half
HM = M // 2                  # j-columns per half (4096)
F32 = mybir.dt.float32
BF16 = mybir.dt.bfloat16

DVE_TILES = {0, 1, 2, 3}     # pairwise row-tiles on VectorE per half
                             # (early tiles — their -wpos is ready first);
                             # ScalarE gets the rest plus the PSUM->SBUF
                             # broadcast copy

_cache = {}


def _build():
    nc = bacc.Bacc("TRN2", target_bir_lowering=False, debug=False,
                   enable_asserts=True, num_devices=N_CORES)
    pos = nc.dram_tensor("pos", [ROWS, L], F32, kind="ExternalInput").ap()
    p = nc.dram_tensor("p", [ROWS, L], F32, kind="ExternalInput").ap()
    neg = nc.dram_tensor("neg", [ROWS, L], F32, kind="ExternalInput").ap()
    out = nc.dram_tensor("partial", [128, 2 * T], F32,
                         kind="ExternalOutput").ap()

    pos_t = pos.rearrange("(t p) l -> t p l", p=128)
    p_t = p.rearrange("(t p) l -> t p l", p=128)
    neg_t = neg.rearrange("(t p) l -> t p l", p=128)

    with tile.TileContext(nc) as tc:
        with tc.tile_pool(name="negp", bufs=T) as neg_pool, \
             tc.tile_pool(name="posp", bufs=3) as pos_pool, \
             tc.tile_pool(name="scrp", bufs=2) as scr_pool, \
             tc.tile_pool(name="big", bufs=1) as big_pool, \
             tc.tile_pool(name="small", bufs=1) as small_pool, \
             tc.tile_pool(name="psum", bufs=1, space="PSUM") as psum_pool, \
             tc.tile_pool(name="dram", bufs=1, space="DRAM") as dpool:

            # warm up the gpsimd SWDGE DMA path (Q7 ucode IRAM load)
            warm = small_pool.tile([1, 16], F32)
            nc.gpsimd.dma_start(warm[:], neg[0:1, 0:16])

            # all-ones column for the TensorE partition-broadcast
            ones_bf = small_pool.tile([1, 128], BF16)
            nc.vector.memset(ones_bf[:], 1.0)
            zeros = small_pool.tile([128, 1], F32)
            nc.vector.memset(zeros[:], 0.0)
            zeros_b = zeros[:].broadcast_to((128, HM))

            # ---- Phase 1: staggered neg DMA + row-max ------------------
            negmax_sb = small_pool.tile([128, T], F32)
            neg_dmas = []
            for t in range(T):
                ntile = neg_pool.tile([128, L], F32, tag="neg")
                ring = nc.sync if t % 2 == 0 else nc.scalar
                d = ring.dma_start(ntile[:], neg_t[t])
                if t >= 2:
                    add_dep_helper(d.ins, neg_dmas[t - 2], sync=True,
                                   reason="stagger neg arrivals")
                neg_dmas.append(d.ins)
                nc.vector.tensor_reduce(negmax_sb[:, t:t + 1], ntile[:],
                                        axis=mybir.AxisListType.X,
                                        op=mybir.AluOpType.max)
                if t == TH - 1:
                    # AG1 bounce-in as soon as tiles 0..3 are reduced
                    negmax1a = small_pool.tile([128, TH], BF16)
                    nc.vector.tensor_scalar_add(negmax1a[:],
                                                negmax_sb[:, :TH], 1.0)
                    cc_in_a = dpool.tile([128, TH], BF16)
                    cc_out_a = dpool.tile([128 * N_CORES, TH], BF16)
                    nc.gpsimd.dma_start(cc_in_a[:], negmax1a[:])

            negmax1b = small_pool.tile([128, TH], BF16)
            nc.vector.tensor_scalar_add(negmax1b[:], negmax_sb[:, TH:], 1.0)
            cc_in_b = dpool.tile([128, TH], BF16)
            cc_out_b = dpool.tile([128 * N_CORES, TH], BF16)
            nc.gpsimd.dma_start(cc_in_b[:], negmax1b[:])

            # ---- Phase 2: the two AllGathers (gpsimd, in order) --------
            nc.gpsimd.collective_compute(
                "AllGather", mybir.AluOpType.bypass,
                ins=[cc_in_a[:].opt()], outs=[cc_out_a[:].opt()],
                replica_groups=[list(range(N_CORES))])
            # gpsimd is blocked until AG1 completes; fetch the gathered
            # row right away (tiny SWDGE DMA)
            negrow_a = small_pool.tile([1, HM], BF16)
            nc.gpsimd.dma_start(
                negrow_a[:],
                cc_out_a[:].rearrange("a b -> (a b)")
                .rearrange("(a b) -> a b", a=1))
            nc.gpsimd.collective_compute(
                "AllGather", mybir.AluOpType.bypass,
                ins=[cc_in_b[:].opt()], outs=[cc_out_b[:].opt()],
                replica_groups=[list(range(N_CORES))])
            negrow_b = small_pool.tile([1, HM], BF16)
            nc.gpsimd.dma_start(
                negrow_b[:],
                cc_out_b[:].rearrange("a b -> (a b)")
                .rearrange("(a b) -> a b", a=1))

            # ---- Phase 3: a = -wpos per row-tile (fused on DVE) --------
            # scalar_tensor_tensor: out = (pos * -1) * p, accum = sum(out)
            a_sb = small_pool.tile([128, T], F32)
            for t in range(T):
                ptile = pos_pool.tile([128, L], F32, tag="pos")
                wtile = pos_pool.tile([128, L], F32, tag="p")
                d0 = nc.sync.dma_start(ptile[:], pos_t[t])
                d1 = nc.scalar.dma_start(wtile[:], p_t[t])
                for nd in neg_dmas:
                    add_dep_helper(d0.ins, nd, sync=True,
                                   reason="neg DMA priority")
                    add_dep_helper(d1.ins, nd, sync=True,
                                   reason="neg DMA priority")
                scr = scr_pool.tile([128, L], BF16, tag="wpos_scr")
                nc.vector.scalar_tensor_tensor(
                    out=scr[:], in0=ptile[:], scalar=-1.0, in1=wtile[:],
                    op0=mybir.AluOpType.mult, op1=mybir.AluOpType.mult,
                    accum_out=a_sb[:, t:t + 1])

            # ---- Phase 4: per j-half: PE broadcast into PSUM, one ACT --
            # ---- copy to SBUF, then pairwise hinge on ACT + DVE --------
            acc = small_pool.tile([128, 2 * T], F32)
            act_scr = big_pool.tile([128, HM], BF16, tag="act_scr")
            dve_scr = big_pool.tile([128, HM], BF16, tag="dve_scr")
            for h, negrow in ((0, negrow_a), (1, negrow_b)):
                psum_bc = psum_pool.tile([128, HM], F32, tag="psum_bc")
                for k in range(HM // 512):
                    nc.tensor.matmul(
                        psum_bc[:, k * 512:(k + 1) * 512],
                        ones_bf[:], negrow[:, k * 512:(k + 1) * 512],
                        start=True, stop=True)
                # single PSUM consumer (PSUM readers serialize); both
                # pairwise engines then read the SBUF copy in parallel
                sb_bc = big_pool.tile([128, HM], BF16, tag=f"sb_bc{h}")
                nc.scalar.copy(sb_bc[:], psum_bc[:])
                for t in range(T):
                    u = h * T + t
                    uc = slice(u, u + 1)
                    if t not in DVE_TILES:
                        nc.scalar.activation(
                            act_scr[:], sb_bc[:],
                            mybir.ActivationFunctionType.Relu,
                            bias=a_sb[:, t:t + 1], scale=1.0,
                            accum_out=acc[:, uc])
                    else:
                        nc.vector.scalar_tensor_tensor(
                            out=dve_scr[:], in0=sb_bc[:],
                            scalar=a_sb[:, t:t + 1], in1=zeros_b,
                            op0=mybir.AluOpType.add, op1=mybir.AluOpType.max,
                            accum_out=acc[:, uc])

            nc.sync.dma_start(out, acc[:])
    nc.compile()
    return nc


def kernel(pos_outputs: np.ndarray, neg_outputs: np.ndarray,
           p: np.ndarray) -> np.ndarray:
    if "nc" not in _cache:
        _cache["nc"] = _build()
    nc = _cache["nc"]

    pos_outputs = np.ascontiguousarray(pos_outputs, dtype=np.float32)
    neg_outputs = np.ascontiguousarray(neg_outputs, dtype=np.float32)
    p = np.ascontiguousarray(p, dtype=np.float32)

    in_maps = []
    for c in range(N_CORES):
        sl = slice(c * ROWS, (c + 1) * ROWS)
        in_maps.append({
            "pos": pos_outputs[sl],
            "p": p[sl],
            "neg": neg_outputs[sl],
        })
    res = bass_utils.run_bass_kernel_spmd(nc, in_maps,
                                          core_ids=list(range(N_CORES)))
    total = 0.0
    for c in range(N_CORES):
        total += res.results[c]["partial"].astype(np.float64).sum()
    return np.asarray(total / (float(N) * float(M)), dtype=np.float32)


# revision 19
# speedup vs baseline: 1.5209x; 1.0538x over previous
"""Trainium2 Bass kernel for nn_CustomLossMinMax.

Computes, over full inputs pos_outputs [N,L], neg_outputs [M,L], p [N,L]
(N=M=8192, L=2048, f32):

    wpos[i]   = sum_l pos[i,l] * p[i,l]
    negmax[j] = max_l neg[j,l]
    out       = sum_ij relu(1 - wpos[i] + negmax[j]) / (N*M)

Sharding (8 cores): rows of pos/p and rows of neg are split 1024/core.
Each core computes its wpos shard and negmax shard, AllGathers the tiny
(1 + negmax) vector in bf16, replicates it across partitions, then
accumulates the pairwise hinge for its own 1024 i-rows against all 8192
j's. Per-core partial sums [128, 16] are summed on the host (the scalar
all-reduce step) and scaled by 1/(N*M).

Schedule per core (DMA of 24 MiB inputs is the roofline; the AllGather
start also absorbs the unavoidable inter-core launch skew):
  - neg tiles stream first (2-deep staggered DMA chain across both
    HWDGE rings); pos/p DMAs carry dep edges on all neg DMAs so the
    AllGather triggers as early as possible.
  - The negmax AllGather is split in TWO: AG1 ships tiles 0-3 as soon
    as they are reduced (~30 us), AG2 ships tiles 4-7. AG1 absorbs the
    inter-core start skew; AG2 pays only the mesh latency. Pairwise
    work on the first 4096 j's overlaps AG2 and the pos/p stream.
  - Partition broadcast per half: TensorE ones-matmuls (ones[1,128].T @
    negrow[1,512]) fill PSUM, then ONE ScalarE copy bridges PSUM->SBUF
    (PSUM readers serialize, so a single reader; ACT and DVE then run
    the pairwise in parallel from SBUF).
  - A tiny gpsimd DMA at kernel start pre-loads the Q7 SWDGE ucode so
    the collective bounce DMAs don't pay its ~8 us cold start.
  - DVE: row-max (8x), fused -(pos*p) row-sum (8x), 8 pairwise units.
  - ACT: 2 broadcast copies + 8 pairwise units with accumulation.

All i/j orderings inside the kernel are permutations of the reference
ordering; the final scalar sum is permutation-invariant. The bf16
transport of (1+negmax) perturbs the result by ~5e-6 relative.
"""
import sys
import numpy as np

for _p in ("/opt/trn_rl_repo", "/root/.axon_site/_ro/trn_rl_repo"):
    if _p not in sys.path:
        sys.path.insert(0, _p)

from concourse import bacc, mybir, tile  # noqa: E402
from concourse import bass_utils  # noqa: E402
from concourse.tile_rust import add_dep_helper  # noqa: E402

N_CORES = 8
N, M, L = 8192, 8192, 2048
ROWS = N // N_CORES          # 1024 rows per core for pos/p and neg
T = ROWS // 128              # 8 row-tiles of 128 partitions per core
TH = T // 2                  # tiles per AllGather half
HM = M // 2                  # j-columns per half (4096)
F32 = mybir.dt.float32
BF16 = mybir.dt.bfloat16

DVE_TILES = {0, 1, 2, 3}     # pairwise row-tiles on VectorE per half
                             # (early tiles — their -wpos is ready first);
                             # ScalarE gets the rest plus the PSUM->SBUF
                             # broadcast copy

_cache = {}


def _build():
    nc = bacc.Bacc("TRN2", target_bir_lowering=False, debug=False,
                   enable_asserts=True, num_devices=N_CORES)
    pos = nc.dram_tensor("pos", [ROWS, L], F32, kind="ExternalInput").ap()
    p = nc.dram_tensor("p", [ROWS, L], F32, kind="ExternalInput").ap()
    neg = nc.dram_tensor("neg", [ROWS, L], F32, kind="ExternalInput").ap()
    out = nc.dram_tensor("partial", [128, 2 * T], F32,
                         kind="ExternalOutput").ap()

    pos_t = pos.rearrange("(t p) l -> t p l", p=128)
    p_t = p.rearrange("(t p) l -> t p l", p=128)
    neg_t = neg.rearrange("(t p) l -> t p l", p=128)

    with tile.TileContext(nc) as tc:
        with tc.tile_pool(name="negp", bufs=T) as neg_pool, \
             tc.tile_pool(name="posp", bufs=3) as pos_pool, \
             tc.tile_pool(name="scrp", bufs=2) as scr_pool, \
             tc.tile_pool(name="big", bufs=1) as big_pool, \
             tc.tile_pool(name="small", bufs=1) as small_pool, \
             tc.tile_pool(name="psum", bufs=1, space="PSUM") as psum_pool, \
             tc.tile_pool(name="dram", bufs=1, space="DRAM") as dpool:

            # warm up the gpsimd SWDGE DMA path (Q7 ucode IRAM load)
            warm = small_pool.tile([1, 16], F32)
            nc.gpsimd.dma_start(warm[:], neg[0:1, 0:16])

            # all-ones column for the TensorE partition-broadcast
            ones_bf = small_pool.tile([1, 128], BF16)
            nc.vector.memset(ones_bf[:], 1.0)
            zeros = small_pool.tile([128, 1], F32)
            nc.vector.memset(zeros[:], 0.0)
            zeros_b = zeros[:].broadcast_to((128, HM))

            # ---- Phase 1: staggered neg DMA + row-max ------------------
            negmax_sb = small_pool.tile([128, T], F32)
            neg_dmas = []
            for t in range(T):
                ntile = neg_pool.tile([128, L], F32, tag="neg")
                ring = nc.sync if t % 2 == 0 else nc.scalar
                d = ring.dma_start(ntile[:], neg_t[t])
                if t >= 2:
                    add_dep_helper(d.ins, neg_dmas[t - 2], sync=True,
                                   reason="stagger neg arrivals")
                neg_dmas.append(d.ins)
                nc.vector.tensor_reduce(negmax_sb[:, t:t + 1], ntile[:],
                                        axis=mybir.AxisListType.X,
                                        op=mybir.AluOpType.max)
                if t == TH - 1:
                    # AG1 bounce-in as soon as tiles 0..3 are reduced
                    negmax1a = small_pool.tile([128, TH], BF16)
                    nc.vector.tensor_scalar_add(negmax1a[:],
                                                negmax_sb[:, :TH], 1.0)
                    cc_in_a = dpool.tile([128, TH], BF16)
                    cc_out_a = dpool.tile([128 * N_CORES, TH], BF16)
                    nc.gpsimd.dma_start(cc_in_a[:], negmax1a[:])

            negmax1b = small_pool.tile([128, TH], BF16)
            nc.vector.tensor_scalar_add(negmax1b[:], negmax_sb[:, TH:], 1.0)
            cc_in_b = dpool.tile([128, TH], BF16)
            cc_out_b = dpool.tile([128 * N_CORES, TH], BF16)
            nc.gpsimd.dma_start(cc_in_b[:], negmax1b[:])

            # ---- Phase 2: the two AllGathers (gpsimd, in order) --------
            nc.gpsimd.collective_compute(
                "AllGather", mybir.AluOpType.bypass,
                ins=[cc_in_a[:].opt()], outs=[cc_out_a[:].opt()],
                replica_groups=[list(range(N_CORES))])
            # gpsimd is blocked until AG1 completes; fetch the gathered
            # row right away (tiny SWDGE DMA)
            negrow_a = small_pool.tile([1, HM], BF16)
            nc.gpsimd.dma_start(
                negrow_a[:],
                cc_out_a[:].rearrange("a b -> (a b)")
                .rearrange("(a b) -> a b", a=1))
            nc.gpsimd.collective_compute(
                "AllGather", mybir.AluOpType.bypass,
                ins=[cc_in_b[:].opt()], outs=[cc_out_b[:].opt()],
                replica_groups=[list(range(N_CORES))])
            negrow_b = small_pool.tile([1, HM], BF16)
            nc.gpsimd.dma_start(
                negrow_b[:],
                cc_out_b[:].rearrange("a b -> (a b)")
                .rearrange("(a b) -> a b", a=1))

            # ---- Phase 3: a = -wpos per row-tile (fused on DVE) --------
            # scalar_tensor_tensor: out = (pos * -1) * p, accum = sum(out)
            a_sb = small_pool.tile([128, T], F32)
            for t in range(T):
                ptile = pos_pool.tile([128, L], F32, tag="pos")
                wtile = pos_pool.tile([128, L], F32, tag="p")
                d0 = nc.sync.dma_start(ptile[:], pos_t[t])
                d1 = nc.scalar.dma_start(wtile[:], p_t[t])
                for nd in neg_dmas:
                    add_dep_helper(d0.ins, nd, sync=True,
                                   reason="neg DMA priority")
                    add_dep_helper(d1.ins, nd, sync=True,
                                   reason="neg DMA priority")
                scr = scr_pool.tile([128, L], BF16, tag="wpos_scr")
                nc.vector.scalar_tensor_tensor(
                    out=scr[:], in0=ptile[:], scalar=-1.0, in1=wtile[:],
                    op0=mybir.AluOpType.mult, op1=mybir.AluOpType.mult,
                    accum_out=a_sb[:, t:t + 1])

            # ---- Phase 4: per j-half: PE broadcast into PSUM, one ACT --
            # ---- copy to SBUF, then pairwise hinge on ACT + DVE --------
            acc = small_pool.tile([128, 2 * T], F32)
            act_scr = big_pool.tile([128, HM], BF16, tag="act_scr")
            dve_scr = big_pool.tile([128, HM], BF16, tag="dve_scr")
            for h, negrow in ((0, negrow_a), (1, negrow_b)):
                psum_bc = psum_pool.tile([128, HM], F32, tag="psum_bc")
                for k in range(HM // 512):
                    nc.tensor.matmul(
                        psum_bc[:, k * 512:(k + 1) * 512],
                        ones_bf[:], negrow[:, k * 512:(k + 1) * 512],
                        start=True, stop=True)
                # single PSUM consumer (PSUM readers serialize); both
                # pairwise engines then read the SBUF copy in parallel
                sb_bc = big_pool.tile([128, HM], BF16, tag=f"sb_bc{h}")
                nc.scalar.copy(sb_bc[:], psum_bc[:])
                for t in range(T):
                    u = h * T + t
                    uc = slice(u, u + 1)
                    if t not in DVE_TILES:
                        nc.scalar.activation(
                            act_scr[:], sb_bc[:],
                            mybir.ActivationFunctionType.Relu,
                            bias=a_sb[:, t:t + 1], scale=1.0,
                            accum_out=acc[:, uc])
                    else:
                        nc.vector.scalar_tensor_tensor(
                            out=dve_scr[:], in0=sb_bc[:],
                            scalar=a_sb[:, t:t + 1], in1=zeros_b,
                            op0=mybir.AluOpType.add, op1=mybir.AluOpType.max,
                            accum_out=acc[:, uc])

            nc.sync.dma_start(out, acc[:])
    nc.compile()
    return nc


def kernel(pos_outputs: np.ndarray, neg_outputs: np.ndarray,
           p: np.ndarray) -> np.ndarray:
    if "nc" not in _cache:
        _cache["nc"] = _build()
    nc = _cache["nc"]

    pos_outputs = np.ascontiguousarray(pos_outputs, dtype=np.float32)
    neg_outputs = np.ascontiguousarray(neg_outputs, dtype=np.float32)
    p = np.ascontiguousarray(p, dtype=np.float32)

    in_maps = []
    for c in range(N_CORES):
        sl = slice(c * ROWS, (c + 1) * ROWS)
        in_maps.append({
            "pos": pos_outputs[sl],
            "p": p[sl],
            "neg": neg_outputs[sl],
        })
    res = bass_utils.run_bass_kernel_spmd(nc, in_maps,
                                          core_ids=list(range(N_CORES)))
    total = 0.0
    for c in range(N_CORES):
        total += res.results[c]["partial"].astype(np.float64).sum()
    return np.asarray(total / (float(N) * float(M)), dtype=np.float32)


# revision 21
# speedup vs baseline: 1.5429x; 1.0145x over previous
"""Trainium2 Bass kernel for nn_CustomLossMinMax.

Computes, over full inputs pos_outputs [N,L], neg_outputs [M,L], p [N,L]
(N=M=8192, L=2048, f32):

    wpos[i]   = sum_l pos[i,l] * p[i,l]
    negmax[j] = max_l neg[j,l]
    out       = sum_ij relu(1 - wpos[i] + negmax[j]) / (N*M)

Sharding (8 cores): rows of pos/p and rows of neg are split 1024/core.
Each core computes its wpos shard and negmax shard, AllGathers the tiny
(1 + negmax) vector in bf16, replicates it across partitions, then
accumulates the pairwise hinge for its own 1024 i-rows against all 8192
j's. Per-core partial sums [128, 16] are summed on the host (the scalar
all-reduce step) and scaled by 1/(N*M).

Schedule per core (DMA of 24 MiB inputs is the roofline; the AllGather
start also absorbs the unavoidable inter-core launch skew):
  - neg tiles stream first (2-deep staggered DMA chain across both
    HWDGE rings); pos/p DMAs carry dep edges on all neg DMAs so the
    AllGather triggers as early as possible.
  - The negmax AllGather is split in TWO: AG1 ships tiles 0-3 as soon
    as they are reduced (~30 us), AG2 ships tiles 4-7. AG1 absorbs the
    inter-core start skew; AG2 pays only the mesh latency. Pairwise
    work on the first 4096 j's overlaps AG2 and the pos/p stream.
  - Partition broadcast per half: TensorE ones-matmuls (ones[1,128].T @
    negrow[1,512]) fill PSUM, then ONE ScalarE copy bridges PSUM->SBUF
    (PSUM readers serialize, so a single reader; ACT and DVE then run
    the pairwise in parallel from SBUF).
  - A tiny gpsimd DMA at kernel start pre-loads the Q7 SWDGE ucode so
    the collective bounce DMAs don't pay its ~8 us cold start.
  - DVE: row-max (8x), fused -(pos*p) row-sum (8x), 8 pairwise units.
  - ACT: 2 broadcast copies + 8 pairwise units with accumulation.

All i/j orderings inside the kernel are permutations of the reference
ordering; the final scalar sum is permutation-invariant. The bf16
transport of (1+negmax) perturbs the result by ~5e-6 relative.
"""
import sys
import numpy as np

for _p in ("/opt/trn_rl_repo", "/root/.axon_site/_ro/trn_rl_repo"):
    if _p not in sys.path:
        sys.path.insert(0, _p)

from concourse import bacc, mybir, tile  # noqa: E402
from concourse import bass_utils  # noqa: E402
from concourse.tile_rust import add_dep_helper  # noqa: E402

N_CORES = 8
N, M, L = 8192, 8192, 2048
ROWS = N // N_CORES          # 1024 rows per core for pos/p and neg
T = ROWS // 128              # 8 row-tiles of 128 partitions per core
TH = T // 2                  # tiles per AllGather half
HM = M // 2                  # j-columns per half (4096)
F32 = mybir.dt.float32
BF16 = mybir.dt.bfloat16

DVE_TILES = {0, 1, 2, 3}     # pairwise row-tiles on VectorE per half
                             # (early tiles — their -wpos is ready first);
                             # ScalarE gets the rest plus the PSUM->SBUF
                             # broadcast copy

_cache = {}


def _build():
    nc = bacc.Bacc("TRN2", target_bir_lowering=False, debug=False,
                   enable_asserts=True, num_devices=N_CORES)
    pos = nc.dram_tensor("pos", [ROWS, L], F32, kind="ExternalInput").ap()
    p = nc.dram_tensor("p", [ROWS, L], F32, kind="ExternalInput").ap()
    neg = nc.dram_tensor("neg", [ROWS, L], F32, kind="ExternalInput").ap()
    out = nc.dram_tensor("partial", [128, 2 * T], F32,
                         kind="ExternalOutput").ap()

    pos_t = pos.rearrange("(t p) l -> t p l", p=128)
    p_t = p.rearrange("(t p) l -> t p l", p=128)
    neg_t = neg.rearrange("(t p) l -> t p l", p=128)

    with tile.TileContext(nc) as tc:
        with tc.tile_pool(name="negp", bufs=T) as neg_pool, \
             tc.tile_pool(name="posp", bufs=3) as pos_pool, \
             tc.tile_pool(name="scrp", bufs=2) as scr_pool, \
             tc.tile_pool(name="big", bufs=1) as big_pool, \
             tc.tile_pool(name="small", bufs=1) as small_pool, \
             tc.tile_pool(name="psum", bufs=1, space="PSUM") as psum_pool, \
             tc.tile_pool(name="dram", bufs=1, space="DRAM") as dpool:

            # warm up the gpsimd SWDGE DMA path (Q7 ucode IRAM load)
            warm = small_pool.tile([1, 16], F32)
            nc.gpsimd.dma_start(warm[:], neg[0:1, 0:16])

            # all-ones column for the TensorE partition-broadcast
            ones_bf = small_pool.tile([1, 128], BF16)
            nc.vector.memset(ones_bf[:], 1.0)
            zeros = small_pool.tile([128, 1], F32)
            nc.vector.memset(zeros[:], 0.0)
            zeros_b = zeros[:].broadcast_to((128, HM))

            # ---- Phase 1: staggered neg DMA + row-max ------------------
            negmax_sb = small_pool.tile([128, T], F32)
            neg_dmas = []
            for t in range(T):
                ntile = neg_pool.tile([128, L], F32, tag="neg")
                ring = nc.sync if t % 2 == 0 else nc.scalar
                d = ring.dma_start(ntile[:], neg_t[t])
                if t >= 2:
                    add_dep_helper(d.ins, neg_dmas[t - 2], sync=True,
                                   reason="stagger neg arrivals")
                neg_dmas.append(d.ins)
                nc.vector.tensor_reduce(negmax_sb[:, t:t + 1], ntile[:],
                                        axis=mybir.AxisListType.X,
                                        op=mybir.AluOpType.max)
                if t == TH - 1:
                    # AG1 bounce-in as soon as tiles 0..3 are reduced
                    negmax1a = small_pool.tile([128, TH], BF16)
                    nc.vector.tensor_scalar_add(negmax1a[:],
                                                negmax_sb[:, :TH], 1.0)
                    cc_in_a = dpool.tile([128, TH], BF16)
                    cc_out_a = dpool.tile([128 * N_CORES, TH], BF16)
                    nc.gpsimd.dma_start(cc_in_a[:], negmax1a[:])

            negmax1b = small_pool.tile([128, TH], BF16)
            nc.vector.tensor_scalar_add(negmax1b[:], negmax_sb[:, TH:], 1.0)
            cc_in_b = dpool.tile([128, TH], BF16)
            cc_out_b = dpool.tile([128 * N_CORES, TH], BF16)
            nc.gpsimd.dma_start(cc_in_b[:], negmax1b[:])

            # ---- Phase 2: the two AllGathers (gpsimd, in order) --------
            nc.gpsimd.collective_compute(
                "AllGather", mybir.AluOpType.bypass,
                ins=[cc_in_a[:].opt()], outs=[cc_out_a[:].opt()],
                replica_groups=[list(range(N_CORES))])
            # gpsimd is blocked until AG1 completes; fetch the gathered
            # row right away (tiny SWDGE DMA)
            negrow_a = small_pool.tile([1, HM], BF16)
            nc.gpsimd.dma_start(
                negrow_a[:],
                cc_out_a[:].rearrange("a b -> (a b)")
                .rearrange("(a b) -> a b", a=1))
            nc.gpsimd.collective_compute(
                "AllGather", mybir.AluOpType.bypass,
                ins=[cc_in_b[:].opt()], outs=[cc_out_b[:].opt()],
                replica_groups=[list(range(N_CORES))])
            negrow_b = small_pool.tile([1, HM], BF16)
            nc.gpsimd.dma_start(
                negrow_b[:],
                cc_out_b[:].rearrange("a b -> (a b)")
                .rearrange("(a b) -> a b", a=1))

            # ---- Phase 3: a = -wpos per row-tile (fused on DVE) --------
            # scalar_tensor_tensor: out = (pos * -1) * p, accum = sum(out)
            a_sb = small_pool.tile([128, T], F32)
            for t in range(T):
                ptile = pos_pool.tile([128, L], F32, tag="pos")
                wtile = pos_pool.tile([128, L], F32, tag="p")
                d0 = nc.sync.dma_start(ptile[:], pos_t[t])
                d1 = nc.scalar.dma_start(wtile[:], p_t[t])
                for nd in neg_dmas:
                    add_dep_helper(d0.ins, nd, sync=True,
                                   reason="neg DMA priority")
                    add_dep_helper(d1.ins, nd, sync=True,
                                   reason="neg DMA priority")
                scr = scr_pool.tile([128, L], BF16, tag="wpos_scr")
                nc.vector.scalar_tensor_tensor(
                    out=scr[:], in0=ptile[:], scalar=-1.0, in1=wtile[:],
                    op0=mybir.AluOpType.mult, op1=mybir.AluOpType.mult,
                    accum_out=a_sb[:, t:t + 1])

            # ---- Phase 4: per j-half: PE broadcast into PSUM, one ACT --
            # ---- copy to SBUF, then pairwise hinge on ACT + DVE --------
            acc = small_pool.tile([128, 2 * T], F32)
            act_scr = big_pool.tile([128, HM], BF16, tag="act_scr")
            dve_scr = big_pool.tile([128, HM], BF16, tag="dve_scr")
            for h, negrow in ((0, negrow_a), (1, negrow_b)):
                psum_bc = psum_pool.tile([128, HM], F32, tag="psum_bc")
                for k in range(HM // 512):
                    nc.tensor.matmul(
                        psum_bc[:, k * 512:(k + 1) * 512],
                        ones_bf[:], negrow[:, k * 512:(k + 1) * 512],
                        start=True, stop=True)
                # single PSUM consumer (PSUM readers serialize); both
                # pairwise engines then read the SBUF copy in parallel
                sb_bc = big_pool.tile([128, HM], BF16, tag=f"sb_bc{h}")
                nc.scalar.copy(sb_bc[:], psum_bc[:])
                for t in range(T):
                    u = h * T + t
                    uc = slice(u, u + 1)
                    if t not in DVE_TILES:
                        nc.scalar.activation(
                            act_scr[:], sb_bc[:],
                            mybir.ActivationFunctionType.Relu,
                            bias=a_sb[:, t:t + 1], scale=1.0,
                            accum_out=acc[:, uc])
                    else:
                        nc.vector.scalar_tensor_tensor(
                            out=dve_scr[:], in0=sb_bc[:],
                            scalar=a_sb[:, t:t + 1], in1=zeros_b,
                            op0=mybir.AluOpType.add, op1=mybir.AluOpType.max,
                            accum_out=acc[:, uc])

            nc.sync.dma_start(out, acc[:])
    nc.compile()
    return nc


def kernel(pos_outputs: np.ndarray, neg_outputs: np.ndarray,
           p: np.ndarray) -> np.ndarray:
    if "nc" not in _cache:
        _cache["nc"] = _build()
    nc = _cache["nc"]

    pos_outputs = np.ascontiguousarray(pos_outputs, dtype=np.float32)
    neg_outputs = np.ascontiguousarray(neg_outputs, dtype=np.float32)
    p = np.ascontiguousarray(p, dtype=np.float32)

    in_maps = []
    for c in range(N_CORES):
        sl = slice(c * ROWS, (c + 1) * ROWS)
        in_maps.append({
            "pos": pos_outputs[sl],
            "p": p[sl],
            "neg": neg_outputs[sl],
        })
    res = bass_utils.run_bass_kernel_spmd(nc, in_maps,
                                          core_ids=list(range(N_CORES)))
    total = 0.0
    for c in range(N_CORES):
        total += res.results[c]["partial"].astype(np.float64).sum()
    return np.asarray(total / (float(N) * float(M)), dtype=np.float32)
